# revision 26
# baseline (speedup 1.0000x reference)
"""Trainium2 Bass kernel: 7x7 valid cross-correlation + bias on a 4096x4096 f32 image.

Formulation: banded matmul on the TensorEngine, in TRANSPOSED orientation
(SBUF partitions = image columns, free dim = image rows).
  out[r, c] = sum_{di,dj} w[di,dj] * x[r+di, c+dj]
For a column-strip of M=122 output columns starting at cb (K = 128 input
columns), with Xb[k, r] = x[r, cb+k]:
  outT[m, r] = sum_di sum_k A_di[k, m] * Xb[k, r+di]
where A_di[k, m] = w[di, k-m] for 0 <= k-m < 7 (banded [128, 122] matrices
precomputed on host). The 7 di-terms accumulate into one PSUM bank via
row-shifted slices of the same SBUF tile (shift in the free dim).

Sharding: output ROWS are split across the 8 cores (512 rows/core + 6 halo);
each core processes all 34 global column strips. Kernel + bias replicated.

DMA layout: the host pre-packs each core's input as
  xsp[p, s*518 + rr] = x[512*c + rr, 122*s + p]
so each SBUF partition's content is contiguous in DRAM -> the whole ~9MB
input loads with 128 descriptors of 8-16KB per instruction (few instructions,
huge descriptors) instead of thousands of 2KB row descriptors. Output is
written transposed (outT[col, row]) for the same reason; the host transposes
back. Matmul operands use float32r (4x fp32 stream rate at N>=256).
"""

import numpy as np

H, W = 4096, 4096
KH, KW = 7, 7
OH, OW = H - KH + 1, W - KW + 1  # 4090, 4090
N_CORES = 8

RPC = 512               # output rows per core
IROWS = RPC + KH - 1    # input rows per core (518)
SW = 122                # output cols per strip (K = SW + KW - 1 = 128)
N_STRIPS = 34           # ceil(4090 / 122) -> covers cols 0..4147 (junk trimmed)
SEG = 518               # packed row-segment length per strip (RPC + KW - 1)
N = 512                 # matmul moving length (rows per strip chunk)
# strips per load DMA (descriptor = chunk*SEG*4 bytes/partition); first chunk
# small so the first matmul's data lands ASAP.
CHUNKS = [2, 4, 4, 4, 4, 4, 4, 4, 4]
# strips per store DMA (descriptor = group*RPC*4 bytes/partition): big groups
# early (fat descriptors drain at full engine rate), tiny groups at the end so
# the last store issues late but drains in ~1us (short tail).
GROUPS = [4, 4, 4, 4, 4, 4, 3, 3, 2, 1, 1]

_cache = {}


def _build_nc():
    import concourse.bacc as bacc
    import concourse.mybir as mybir
    from concourse.tile import TileContext

    f32 = mybir.dt.float32
    bf16 = mybir.dt.bfloat16  # halves DMA bytes; 1 cycle/row matmul stream

    nc = bacc.Bacc("TRN2", target_bir_lowering=False, debug=False)
    xsp = nc.dram_tensor("xsp", [128, N_STRIPS * SEG], bf16, kind="ExternalInput")
    bands = nc.dram_tensor("bands", [128, KH * 128], bf16, kind="ExternalInput")
    biasv = nc.dram_tensor("biasv", [128, 1], f32, kind="ExternalInput")
    # out2[m, s*RPC + r] = out[512*c + r, 122*s + m]: strip-major per partition
    # so each grouped store writes one long contiguous DRAM run per partition.
    out2 = nc.dram_tensor("out2", [SW, N_STRIPS * RPC], bf16, kind="ExternalOutput")

    assert sum(CHUNKS) == N_STRIPS
    n_chunks = len(CHUNKS)
    chunk_of = []  # strip -> (chunk index, offset within chunk, chunk col base)
    base = 0
    for ci, cn in enumerate(CHUNKS):
        for so in range(cn):
            chunk_of.append((ci, so, base))
        base += cn
    assert sum(GROUPS) == N_STRIPS
    n_groups = len(GROUPS)
    group_of = []  # strip -> (group index, offset within group, group col base)
    gbase = 0
    for gi, gn in enumerate(GROUPS):
        for go in range(gn):
            group_of.append((gi, go, gbase))
        gbase += gn

    with TileContext(nc) as tc:
        with (
            tc.tile_pool(name="const", bufs=1) as cpool,
            tc.tile_pool(name="xc", bufs=n_chunks) as xpool,
            tc.tile_pool(name="acc", bufs=n_groups) as apool,
            tc.tile_pool(name="psum", bufs=8, space="PSUM") as ppool,
        ):
            band_t = cpool.tile([128, KH * 128], bf16)
            nc.gpsimd.dma_start(out=band_t[:, :], in_=bands[:, :])
            bias_t = cpool.tile([128, 1], f32)
            nc.gpsimd.dma_start(out=bias_t[:, :], in_=biasv[:, :])

            # All data DMA goes through SWDGE (gpsimd): HWDGE rings
            # unpredictably pin to 2 SDMA engines. Each SWDGE instruction
            # drains on exactly 2 SDMA engines (round-robin per instruction),
            # so split each transfer into partition-sliced instructions to
            # engage more engine pairs in parallel.
            x_ts = []
            s0 = 0
            for ci, ns in enumerate(CHUNKS):
                xt = xpool.tile([128, ns * SEG], bf16, tag="xc")
                nc.gpsimd.dma_start(
                    out=xt[:, :], in_=xsp[:, s0 * SEG : (s0 + ns) * SEG]
                )
                x_ts.append(xt)
                s0 += ns

            acc_ts = [None] * n_groups
            for s in range(N_STRIPS):
                ci, so, _ = chunk_of[s]
                gi, go, g0 = group_of[s]
                xt = x_ts[ci]
                if acc_ts[gi] is None:
                    acc_ts[gi] = apool.tile(
                        [128, GROUPS[gi] * RPC], bf16, name="acc", tag="acc"
                    )
                ps = ppool.tile([128, N], f32, tag="ps")
                for di in range(KH):
                    nc.tensor.matmul(
                        ps[:SW, :],
                        band_t[:, di * 128 : di * 128 + SW],
                        xt[:, so * SEG + di : so * SEG + di + N],
                        start=(di == 0),
                        stop=(di == KH - 1),
                    )
                nc.vector.tensor_scalar_add(
                    acc_ts[gi][:SW, go * RPC : go * RPC + N],
                    ps[:SW, :],
                    bias_t[:SW, :1],
                )
                if go == GROUPS[gi] - 1:
                    gs = GROUPS[gi]
                    # Final two stores ride the (idle-by-then) HWDGE rings so
                    # they drain in parallel with the SWDGE queue's backlog.
                    if gi == n_groups - 1:
                        eng = nc.sync
                    elif gi == n_groups - 2:
                        eng = nc.scalar
                    else:
                        eng = nc.gpsimd
                    eng.dma_start(
                        out=out2[:, g0 * RPC : (g0 + gs) * RPC],
                        in_=acc_ts[gi][:SW, :],
                    )

    nc.finalize()
    return nc


def _get_nc():
    if "nc" not in _cache:
        _cache["nc"] = _build_nc()
    return _cache["nc"]


def _build_bands(weight: np.ndarray) -> np.ndarray:
    """bands[k, di*128 + m] = w[di, k-m] for 0 <= k-m < KW, m < SW."""
    w = np.asarray(weight, np.float32)
    bands = np.zeros((128, KH * 128), np.float32)
    m = np.arange(SW)
    for di in range(KH):
        for dj in range(KW):
            bands[m + dj, di * 128 + m] = w[di, dj]
    return bands


def _prepare_in_maps(x, weight, bias):
    from ml_dtypes import bfloat16

    x = np.asarray(x, np.float32)
    bands = _build_bands(weight).astype(bfloat16)
    bias_tile = np.full((128, 1), np.float32(np.asarray(bias).reshape(-1)[0]))

    in_maps = []
    for c in range(N_CORES):
        r0 = c * RPC
        take = min(IROWS, H - r0)
        xc = np.zeros((IROWS, N_STRIPS * SW + 128 - SW), np.float32)
        xc[:take, :W] = x[r0 : r0 + take, :]
        xsp = np.empty((128, N_STRIPS * SEG), bfloat16)
        for s in range(N_STRIPS):
            xsp[:, s * SEG : (s + 1) * SEG] = xc[:, s * SW : s * SW + 128].T
        in_maps.append({"xsp": xsp, "bands": bands, "biasv": bias_tile})
    return in_maps


def _gather_out(per_core_outs) -> np.ndarray:
    out = np.empty((OH, OW), np.float32)
    for c in range(N_CORES):
        r0 = c * RPC
        take = min(RPC, OH - r0)
        o2 = per_core_outs[c]["out2"].astype(np.float32)
        o2 = o2.reshape(SW, N_STRIPS, RPC)
        cols = o2.transpose(1, 0, 2).reshape(N_STRIPS * SW, RPC)  # [col, row]
        out[r0 : r0 + take, :] = cols[:OW, :take].T
    return out


def kernel(x: np.ndarray, weight: np.ndarray, bias: np.ndarray) -> np.ndarray:
    from concourse import bass_utils

    nc = _get_nc()
    in_maps = _prepare_in_maps(x, weight, bias)
    res = bass_utils.run_bass_kernel_spmd(nc, in_maps, list(range(N_CORES)))
    _cache["last_results"] = res
    return _gather_out(res.results)


# revision 27
# speedup vs baseline: 1.0173x; 1.0173x over previous
"""Trainium2 Bass kernel: 7x7 valid cross-correlation + bias on a 4096x4096 f32 image.

Formulation: banded matmul on the TensorEngine, in TRANSPOSED orientation
(SBUF partitions = image columns, free dim = image rows).
  out[r, c] = sum_{di,dj} w[di,dj] * x[r+di, c+dj]
For a column-strip of M=122 output columns starting at cb (K = 128 input
columns), with Xb[k, r] = x[r, cb+k]:
  outT[m, r] = sum_di sum_k A_di[k, m] * Xb[k, r+di]
where A_di[k, m] = w[di, k-m] for 0 <= k-m < 7 (banded [128, 122] matrices
precomputed on host). The 7 di-terms accumulate into one PSUM bank via
row-shifted slices of the same SBUF tile (shift in the free dim).

Sharding: output ROWS are split across the 8 cores (512 rows/core + 6 halo);
each core processes all 34 global column strips. Kernel + bias replicated.

DMA layout: the host pre-packs each core's input as
  xsp[p, s*518 + rr] = x[512*c + rr, 122*s + p]
so each SBUF partition's content is contiguous in DRAM -> the whole ~9MB
input loads with 128 descriptors of 8-16KB per instruction (few instructions,
huge descriptors) instead of thousands of 2KB row descriptors. Output is
written transposed (outT[col, row]) for the same reason; the host transposes
back. Matmul operands use float32r (4x fp32 stream rate at N>=256).
"""

import numpy as np

H, W = 4096, 4096
KH, KW = 7, 7
OH, OW = H - KH + 1, W - KW + 1  # 4090, 4090
N_CORES = 8

RPC = 512               # output rows per core
IROWS = RPC + KH - 1    # input rows per core (518)
SW = 122                # output cols per strip (K = SW + KW - 1 = 128)
N_STRIPS = 34           # ceil(4090 / 122) -> covers cols 0..4147 (junk trimmed)
SEG = 518               # packed row-segment length per strip (RPC + KW - 1)
N = 512                 # matmul moving length (rows per strip chunk)
# strips per load DMA (descriptor = chunk*SEG*4 bytes/partition); first chunk
# small so the first matmul's data lands ASAP.
CHUNKS = [2, 4, 4, 4, 4, 4, 4, 4, 4]
# strips per store DMA (descriptor = group*RPC*4 bytes/partition): big groups
# early (fat descriptors drain at full engine rate), tiny groups at the end so
# the last store issues late but drains in ~1us (short tail).
GROUPS = [4, 4, 4, 4, 3, 3, 3, 3, 2, 2, 1, 1]

_cache = {}


def _build_nc():
    import concourse.bacc as bacc
    import concourse.mybir as mybir
    from concourse.tile import TileContext

    f32 = mybir.dt.float32
    bf16 = mybir.dt.bfloat16  # halves DMA bytes; 1 cycle/row matmul stream

    nc = bacc.Bacc("TRN2", target_bir_lowering=False, debug=False)
    xsp = nc.dram_tensor("xsp", [128, N_STRIPS * SEG], bf16, kind="ExternalInput")
    bands = nc.dram_tensor("bands", [128, KH * 128], bf16, kind="ExternalInput")
    biasv = nc.dram_tensor("biasv", [128, 1], f32, kind="ExternalInput")
    # out2[m, s*RPC + r] = out[512*c + r, 122*s + m]: strip-major per partition
    # so each grouped store writes one long contiguous DRAM run per partition.
    out2 = nc.dram_tensor("out2", [SW, N_STRIPS * RPC], bf16, kind="ExternalOutput")

    assert sum(CHUNKS) == N_STRIPS
    n_chunks = len(CHUNKS)
    chunk_of = []  # strip -> (chunk index, offset within chunk, chunk col base)
    base = 0
    for ci, cn in enumerate(CHUNKS):
        for so in range(cn):
            chunk_of.append((ci, so, base))
        base += cn
    assert sum(GROUPS) == N_STRIPS
    n_groups = len(GROUPS)
    group_of = []  # strip -> (group index, offset within group, group col base)
    gbase = 0
    for gi, gn in enumerate(GROUPS):
        for go in range(gn):
            group_of.append((gi, go, gbase))
        gbase += gn

    with TileContext(nc) as tc:
        with (
            tc.tile_pool(name="const", bufs=1) as cpool,
            tc.tile_pool(name="xc", bufs=n_chunks) as xpool,
            tc.tile_pool(name="acc", bufs=n_groups) as apool,
            tc.tile_pool(name="psum", bufs=8, space="PSUM") as ppool,
        ):
            band_t = cpool.tile([128, KH * 128], bf16)
            nc.gpsimd.dma_start(out=band_t[:, :], in_=bands[:, :])
            bias_t = cpool.tile([128, 1], f32)
            nc.gpsimd.dma_start(out=bias_t[:, :], in_=biasv[:, :])

            # All data DMA goes through SWDGE (gpsimd): HWDGE rings
            # unpredictably pin to 2 SDMA engines. Each SWDGE instruction
            # drains on exactly 2 SDMA engines (round-robin per instruction),
            # so split each transfer into partition-sliced instructions to
            # engage more engine pairs in parallel.
            x_ts = []
            s0 = 0
            for ci, ns in enumerate(CHUNKS):
                xt = xpool.tile([128, ns * SEG], bf16, tag="xc")
                nc.gpsimd.dma_start(
                    out=xt[:, :], in_=xsp[:, s0 * SEG : (s0 + ns) * SEG]
                )
                x_ts.append(xt)
                s0 += ns

            acc_ts = [None] * n_groups
            for s in range(N_STRIPS):
                ci, so, _ = chunk_of[s]
                gi, go, g0 = group_of[s]
                xt = x_ts[ci]
                if acc_ts[gi] is None:
                    acc_ts[gi] = apool.tile(
                        [128, GROUPS[gi] * RPC], bf16, name="acc", tag="acc"
                    )
                ps = ppool.tile([128, N], f32, tag="ps")
                for di in range(KH):
                    nc.tensor.matmul(
                        ps[:SW, :],
                        band_t[:, di * 128 : di * 128 + SW],
                        xt[:, so * SEG + di : so * SEG + di + N],
                        start=(di == 0),
                        stop=(di == KH - 1),
                    )
                nc.vector.tensor_scalar_add(
                    acc_ts[gi][:SW, go * RPC : go * RPC + N],
                    ps[:SW, :],
                    bias_t[:SW, :1],
                )
                if go == GROUPS[gi] - 1:
                    gs = GROUPS[gi]
                    nc.gpsimd.dma_start(
                        out=out2[:, g0 * RPC : (g0 + gs) * RPC],
                        in_=acc_ts[gi][:SW, :],
                    )

    nc.finalize()
    return nc


def _get_nc():
    if "nc" not in _cache:
        _cache["nc"] = _build_nc()
    return _cache["nc"]


def _build_bands(weight: np.ndarray) -> np.ndarray:
    """bands[k, di*128 + m] = w[di, k-m] for 0 <= k-m < KW, m < SW."""
    w = np.asarray(weight, np.float32)
    bands = np.zeros((128, KH * 128), np.float32)
    m = np.arange(SW)
    for di in range(KH):
        for dj in range(KW):
            bands[m + dj, di * 128 + m] = w[di, dj]
    return bands


def _prepare_in_maps(x, weight, bias):
    from ml_dtypes import bfloat16

    x = np.asarray(x, np.float32)
    bands = _build_bands(weight).astype(bfloat16)
    bias_tile = np.full((128, 1), np.float32(np.asarray(bias).reshape(-1)[0]))

    in_maps = []
    for c in range(N_CORES):
        r0 = c * RPC
        take = min(IROWS, H - r0)
        xc = np.zeros((IROWS, N_STRIPS * SW + 128 - SW), np.float32)
        xc[:take, :W] = x[r0 : r0 + take, :]
        xsp = np.empty((128, N_STRIPS * SEG), bfloat16)
        for s in range(N_STRIPS):
            xsp[:, s * SEG : (s + 1) * SEG] = xc[:, s * SW : s * SW + 128].T
        in_maps.append({"xsp": xsp, "bands": bands, "biasv": bias_tile})
    return in_maps


def _gather_out(per_core_outs) -> np.ndarray:
    out = np.empty((OH, OW), np.float32)
    for c in range(N_CORES):
        r0 = c * RPC
        take = min(RPC, OH - r0)
        o2 = per_core_outs[c]["out2"].astype(np.float32)
        o2 = o2.reshape(SW, N_STRIPS, RPC)
        cols = o2.transpose(1, 0, 2).reshape(N_STRIPS * SW, RPC)  # [col, row]
        out[r0 : r0 + take, :] = cols[:OW, :take].T
    return out


def kernel(x: np.ndarray, weight: np.ndarray, bias: np.ndarray) -> np.ndarray:
    from concourse import bass_utils

    nc = _get_nc()
    in_maps = _prepare_in_maps(x, weight, bias)
    res = bass_utils.run_bass_kernel_spmd(nc, in_maps, list(range(N_CORES)))
    _cache["last_results"] = res
    return _gather_out(res.results)


# revision 28
# speedup vs baseline: 1.0576x; 1.0397x over previous
"""Trainium2 Bass kernel: 7x7 valid cross-correlation + bias on a 4096x4096 f32 image.

Formulation: banded matmul on the TensorEngine, in TRANSPOSED orientation
(SBUF partitions = image columns, free dim = image rows).
  out[r, c] = sum_{di,dj} w[di,dj] * x[r+di, c+dj]
For a column-strip of M=122 output columns starting at cb (K = 128 input
columns), with Xb[k, r] = x[r, cb+k]:
  outT[m, r] = sum_di sum_k A_di[k, m] * Xb[k, r+di]
where A_di[k, m] = w[di, k-m] for 0 <= k-m < 7 (banded [128, 122] matrices
precomputed on host). The 7 di-terms accumulate into one PSUM bank via
row-shifted slices of the same SBUF tile (shift in the free dim).

Sharding: output ROWS are split across the 8 cores (512 rows/core + 6 halo);
each core processes all 34 global column strips. Kernel + bias replicated.

DMA layout: the host pre-packs each core's input as
  xsp[p, s*518 + rr] = x[512*c + rr, 122*s + p]
so each SBUF partition's content is contiguous in DRAM -> the whole ~9MB
input loads with 128 descriptors of 8-16KB per instruction (few instructions,
huge descriptors) instead of thousands of 2KB row descriptors. Output is
written transposed (outT[col, row]) for the same reason; the host transposes
back. Matmul operands use float32r (4x fp32 stream rate at N>=256).
"""

import numpy as np

H, W = 4096, 4096
KH, KW = 7, 7
OH, OW = H - KH + 1, W - KW + 1  # 4090, 4090
N_CORES = 8

RPC = 512               # output rows per core
IROWS = RPC + KH - 1    # input rows per core (518)
SW = 122                # output cols per strip (K = SW + KW - 1 = 128)
N_STRIPS = 34           # ceil(4090 / 122) -> covers cols 0..4147 (junk trimmed)
SEG = 518               # packed row-segment length per strip (RPC + KW - 1)
N = 512                 # matmul moving length (rows per strip chunk)
# strips per load DMA (descriptor = chunk*SEG*4 bytes/partition); first chunk
# small so the first matmul's data lands ASAP.
CHUNKS = [2, 3, 3, 4, 4, 4, 4, 5, 5]
# strips per store DMA (descriptor = group*RPC*4 bytes/partition): big groups
# early (fat descriptors drain at full engine rate), tiny groups at the end so
# the last store issues late but drains in ~1us (short tail).
GROUPS = [4, 4, 4, 4, 4, 4, 3, 3, 2, 1, 1]

_cache = {}


def _build_nc():
    import concourse.bacc as bacc
    import concourse.mybir as mybir
    from concourse.tile import TileContext

    f32 = mybir.dt.float32
    bf16 = mybir.dt.bfloat16  # halves DMA bytes; 1 cycle/row matmul stream

    nc = bacc.Bacc("TRN2", target_bir_lowering=False, debug=False)
    xsp = nc.dram_tensor("xsp", [128, N_STRIPS * SEG], bf16, kind="ExternalInput")
    bands = nc.dram_tensor("bands", [128, KH * 128], bf16, kind="ExternalInput")
    biasv = nc.dram_tensor("biasv", [128, 1], f32, kind="ExternalInput")
    # out2[m, s*RPC + r] = out[512*c + r, 122*s + m]: strip-major per partition
    # so each grouped store writes one long contiguous DRAM run per partition.
    out2 = nc.dram_tensor("out2", [SW, N_STRIPS * RPC], bf16, kind="ExternalOutput")

    assert sum(CHUNKS) == N_STRIPS
    n_chunks = len(CHUNKS)
    chunk_of = []  # strip -> (chunk index, offset within chunk, chunk col base)
    base = 0
    for ci, cn in enumerate(CHUNKS):
        for so in range(cn):
            chunk_of.append((ci, so, base))
        base += cn
    assert sum(GROUPS) == N_STRIPS
    n_groups = len(GROUPS)
    group_of = []  # strip -> (group index, offset within group, group col base)
    gbase = 0
    for gi, gn in enumerate(GROUPS):
        for go in range(gn):
            group_of.append((gi, go, gbase))
        gbase += gn

    with TileContext(nc) as tc:
        with (
            tc.tile_pool(name="const", bufs=1) as cpool,
            tc.tile_pool(name="xc", bufs=n_chunks) as xpool,
            tc.tile_pool(name="acc", bufs=n_groups) as apool,
            tc.tile_pool(name="psum", bufs=8, space="PSUM") as ppool,
        ):
            band_t = cpool.tile([128, KH * 128], bf16)
            nc.gpsimd.dma_start(out=band_t[:, :], in_=bands[:, :])
            bias_t = cpool.tile([128, 1], f32)
            nc.gpsimd.dma_start(out=bias_t[:, :], in_=biasv[:, :])

            # All data DMA goes through SWDGE (gpsimd): HWDGE rings
            # unpredictably pin to 2 SDMA engines. Each SWDGE instruction
            # drains on exactly 2 SDMA engines (round-robin per instruction),
            # so split each transfer into partition-sliced instructions to
            # engage more engine pairs in parallel.
            x_ts = []
            s0 = 0
            for ci, ns in enumerate(CHUNKS):
                xt = xpool.tile([128, ns * SEG], bf16, tag="xc")
                nc.gpsimd.dma_start(
                    out=xt[:, :], in_=xsp[:, s0 * SEG : (s0 + ns) * SEG]
                )
                x_ts.append(xt)
                s0 += ns

            acc_ts = [None] * n_groups
            for s in range(N_STRIPS):
                ci, so, _ = chunk_of[s]
                gi, go, g0 = group_of[s]
                xt = x_ts[ci]
                if acc_ts[gi] is None:
                    acc_ts[gi] = apool.tile(
                        [128, GROUPS[gi] * RPC], bf16, name="acc", tag="acc"
                    )
                ps = ppool.tile([128, N], f32, tag="ps")
                for di in range(KH):
                    nc.tensor.matmul(
                        ps[:SW, :],
                        band_t[:, di * 128 : di * 128 + SW],
                        xt[:, so * SEG + di : so * SEG + di + N],
                        start=(di == 0),
                        stop=(di == KH - 1),
                    )
                nc.vector.tensor_scalar_add(
                    acc_ts[gi][:SW, go * RPC : go * RPC + N],
                    ps[:SW, :],
                    bias_t[:SW, :1],
                )
                if go == GROUPS[gi] - 1:
                    gs = GROUPS[gi]
                    nc.gpsimd.dma_start(
                        out=out2[:, g0 * RPC : (g0 + gs) * RPC],
                        in_=acc_ts[gi][:SW, :],
                    )

    nc.finalize()
    return nc


def _get_nc():
    if "nc" not in _cache:
        _cache["nc"] = _build_nc()
    return _cache["nc"]


def _build_bands(weight: np.ndarray) -> np.ndarray:
    """bands[k, di*128 + m] = w[di, k-m] for 0 <= k-m < KW, m < SW."""
    w = np.asarray(weight, np.float32)
    bands = np.zeros((128, KH * 128), np.float32)
    m = np.arange(SW)
    for di in range(KH):
        for dj in range(KW):
            bands[m + dj, di * 128 + m] = w[di, dj]
    return bands


def _prepare_in_maps(x, weight, bias):
    from ml_dtypes import bfloat16

    x = np.asarray(x, np.float32)
    bands = _build_bands(weight).astype(bfloat16)
    bias_tile = np.full((128, 1), np.float32(np.asarray(bias).reshape(-1)[0]))

    in_maps = []
    for c in range(N_CORES):
        r0 = c * RPC
        take = min(IROWS, H - r0)
        xc = np.zeros((IROWS, N_STRIPS * SW + 128 - SW), np.float32)
        xc[:take, :W] = x[r0 : r0 + take, :]
        xsp = np.empty((128, N_STRIPS * SEG), bfloat16)
        for s in range(N_STRIPS):
            xsp[:, s * SEG : (s + 1) * SEG] = xc[:, s * SW : s * SW + 128].T
        in_maps.append({"xsp": xsp, "bands": bands, "biasv": bias_tile})
    return in_maps


def _gather_out(per_core_outs) -> np.ndarray:
    out = np.empty((OH, OW), np.float32)
    for c in range(N_CORES):
        r0 = c * RPC
        take = min(RPC, OH - r0)
        o2 = per_core_outs[c]["out2"].astype(np.float32)
        o2 = o2.reshape(SW, N_STRIPS, RPC)
        cols = o2.transpose(1, 0, 2).reshape(N_STRIPS * SW, RPC)  # [col, row]
        out[r0 : r0 + take, :] = cols[:OW, :take].T
    return out


def kernel(x: np.ndarray, weight: np.ndarray, bias: np.ndarray) -> np.ndarray:
    from concourse import bass_utils

    nc = _get_nc()
    in_maps = _prepare_in_maps(x, weight, bias)
    res = bass_utils.run_bass_kernel_spmd(nc, in_maps, list(range(N_CORES)))
    _cache["last_results"] = res
    return _gather_out(res.results)
